# revision 43
# baseline (speedup 1.0000x reference)
"""Bahdanau attention kernel for 8 TRN2 NeuronCores.

Reference math (per batch b):
    pq = q @ W_s                          (T, H)
    pe = enc @ W_h                        (S, H)
    score[t,s] = sum_h v[h] * tanh(pq[t,h] + pe[s,h])
    align = softmax_s(score masked by src_len)
    ctx = align @ enc                     (T, H)
    out = tanh([ctx, q] @ W_out)          (T, H)

Sharding: data-parallel over (b, T-half) -> 8 cores, 64 t's per core.
No collectives; each core owns a disjoint output slice.  The host passes
transposed layouts (encT, qT) so no on-device transposes are needed.

Per-core pipeline (H-chunks of 128 on partitions):
    PE:  peT[k, s], pqT[k, t] projections.
    DVE: staging[k, (t,s)] = peT + pqT[:, t]  (per-partition scalar add, 2x)
    ACT: tanh over wide (128, 8192) tiles, emitted bf16   <-- bottleneck
    PE:  scoreT[s, t] columns = tanh_tile(128h x 128s).T @ v_chunk(128x1)
         accumulated over the 4 h-chunks (FWL keeps weight loads cheap);
         masking pre-loads -1e30 into masked s rows via a K=4 matmul.
    ACT: expT = exp(scoreT)   (no max subtraction: |score| stays small)
    PE (overlapped with the t-loop): ap2 = q @ Wbot, EW = enc @ Wtop.
    Tail: denom (ones reduce) -> recip -> ap1 = expT.T @ EW,
          out = tanh(recip[t]*ap1 + ap2).
"""

import sys
from contextlib import ExitStack

import numpy as np

for _p in ("/opt/trn_rl_repo",):
    if _p not in sys.path:
        sys.path.insert(0, _p)

import concourse.bacc as bacc
import concourse.tile as tile
from concourse import mybir
from concourse.bass_utils import run_bass_kernel_spmd

B, T, S, H = 4, 128, 512, 512
NCORES = 8
TC = 64          # t's per core
TG = 4           # t's per tanh batch
NGROUPS = TC // TG
F32 = mybir.dt.float32
BF16 = mybir.dt.bfloat16
AF = mybir.ActivationFunctionType
MASK_NEG = -1.0e30


def _build_kernel(ctx: ExitStack, tc: tile.TileContext, io: dict):
    nc = tc.nc

    st = ctx.enter_context(tc.tile_pool(name="statics", bufs=1))
    stage_pool = ctx.enter_context(tc.tile_pool(name="stage", bufs=3))
    tanh_pool = ctx.enter_context(tc.tile_pool(name="tanhp", bufs=2))
    ps_score = ctx.enter_context(tc.tile_pool(name="ps_score", bufs=1, space="PSUM"))
    ps_big = ctx.enter_context(tc.tile_pool(name="ps_big", bufs=2, space="PSUM"))
    ps_small = ctx.enter_context(tc.tile_pool(name="ps_small", bufs=2, space="PSUM"))

    # ---- static tiles + input DMAs (critical-path first) ----
    encT_sb = st.tile([128, 4 * 512], F32, tag="encT")   # [hc] h-part, s-free
    wh_sb = st.tile([128, 4 * 512], F32, tag="wh")       # [hc] h-part, k-free
    ws_sb = st.tile([128, 4 * 512], F32, tag="ws")       # [hc] h-part, k-free
    wout_sb = st.tile([128, 8 * 512], F32, tag="wout")   # [rc] row-part, o-free
    qT_sb = st.tile([128, 4 * 64], F32, tag="qT")        # [hc] h-part, t-free
    v4_sb = st.tile([128, 4], F32, tag="v4")
    v4_bf = st.tile([128, 4], BF16, tag="v4b")
    mneg_sb = st.tile([4, 128], F32, tag="mneg")         # mneg[sb, s_local]
    blockones = st.tile([4, 256], F32, tag="blockones")

    # spread input DMAs over three issue queues so the projection inputs
    # (qT+ws for pqT, wh+encT for peT) all land within ~4us
    nc.sync.dma_start(v4_sb[:], io["v4"][:])
    for c in range(4):
        nc.sync.dma_start(qT_sb[:, c * 64:(c + 1) * 64], io["qT"][c * 128:(c + 1) * 128, :])
    for c in range(4):
        nc.sync.dma_start(encT_sb[:, c * 512:(c + 1) * 512], io["encT"][c * 128:(c + 1) * 128, :])
    for c in range(4):
        nc.gpsimd.dma_start(ws_sb[:, c * 512:(c + 1) * 512], io["ws"][c * 128:(c + 1) * 128, :])
    for c in range(4):
        nc.scalar.dma_start(wh_sb[:, c * 512:(c + 1) * 512], io["wh"][c * 128:(c + 1) * 128, :])
    nc.gpsimd.dma_start(mneg_sb[:], io["mneg"][:])
    nc.gpsimd.dma_start(blockones[:], io["bones"][:])
    for c in range(8):
        nc.gpsimd.dma_start(wout_sb[:, c * 512:(c + 1) * 512], io["wout"][c * 128:(c + 1) * 128, :])
    nc.vector.tensor_copy(v4_bf[:], v4_sb[:])

    ones_row = st.tile([1, 64], F32, tag="ones_row")
    nc.vector.memset(ones_row[:], 1.0)
    ones_col = st.tile([128, 1], F32, tag="ones_col")
    nc.vector.memset(ones_col[:], 1.0)

    # ---- projections: peT[k,s], pqT[k,t] ----
    peT_sb = st.tile([128, 4 * 512], F32, tag="peT")     # [kc] k-part, s-free
    pqT_sb = st.tile([128, 4 * 64], F32, tag="pqT")      # [kc] k-part, t-free
    for kc in range(4):
        pq = ps_small.tile([128, 64], F32, tag="small")
        for hc in range(4):
            nc.tensor.matmul(pq[:], ws_sb[:, hc * 512 + kc * 128: hc * 512 + (kc + 1) * 128],
                             qT_sb[:, hc * 64:(hc + 1) * 64],
                             start=(hc == 0), stop=(hc == 3))
        nc.vector.tensor_copy(pqT_sb[:, kc * 64:(kc + 1) * 64], pq[:])
        pp = ps_big.tile([128, 512], F32, tag="big")
        for hc in range(4):
            nc.tensor.matmul(pp[:], wh_sb[:, hc * 512 + kc * 128: hc * 512 + (kc + 1) * 128],
                             encT_sb[:, hc * 512:(hc + 1) * 512],
                             start=(hc == 0), stop=(hc == 3))
        nc.vector.tensor_copy(peT_sb[:, kc * 512:(kc + 1) * 512], pp[:])

    # ---- scoreT accumulation in PSUM: (128 s x 64 t) per s-block ----
    scT = ps_score.tile([128, 4 * 64], F32, tag="scT")
    # masked s rows start at -1e30 (exp -> 0), live rows at 0.  One matmul
    # covering the whole tile: start=True clears has_written bank-wide, so
    # this must be a single accumulation-group opener.
    nc.tensor.matmul(scT[:], mneg_sb[:], blockones[:],
                     start=True, stop=False, skip_group_check=True)

    EW_sb = st.tile([128, 4 * 512], F32, tag="EW")       # [sb] s-part, o-free
    ap2_sb = st.tile([64, 512], F32, tag="ap2")

    # Ramp: the first two groups run with their tanh split into per-kc
    # strided sub-ops, interleaved g0/g1, so ACT streams continuously while
    # the peT[kc] projection pipeline is still filling.
    def preadds(stg, t0, W, kc):
        for ti in range(W):
            t = t0 + ti
            nc.vector.tensor_scalar_add(
                stg[:, (ti * 4 + kc) * 512:(ti * 4 + kc + 1) * 512],
                peT_sb[:, kc * 512:(kc + 1) * 512],
                pqT_sb[:, kc * 64 + t: kc * 64 + t + 1])

    def vreduce(th, t0, W):
        for ti in range(W):
            t = t0 + ti
            for sb in range(4):
                for kc in range(4):
                    last = (t == TC - 1 and kc == 3)
                    nc.tensor.matmul(
                        scT[:, sb * 64 + t: sb * 64 + t + 1],
                        th[:, (ti * 4 + kc) * 512 + sb * 128: (ti * 4 + kc) * 512 + (sb + 1) * 128],
                        v4_bf[:, kc:kc + 1],
                        start=False, stop=last, skip_group_check=True)

    ramp_tiles = []
    for r in range(2):
        stg = stage_pool.tile([128, TG * 2048], F32, tag="stg")
        th = tanh_pool.tile([128, TG * 2048], BF16, tag="th")
        ramp_tiles.append((stg, th))
    for kc in range(4):
        for r in range(2):
            stg, th = ramp_tiles[r]
            preadds(stg, r * TG, TG, kc)
            stg_v = stg[:].rearrange("p (ti kc f) -> p ti kc f", kc=4, f=512)
            th_v = th[:].rearrange("p (ti kc f) -> p ti kc f", kc=4, f=512)
            nc.scalar.activation(th_v[:, :, kc, :], stg_v[:, :, kc, :], AF.Tanh)
    for r in range(2):
        vreduce(ramp_tiles[r][1], r * TG, TG)

    for g in range(2, NGROUPS):
        t0 = g * TG
        stg = stage_pool.tile([128, TG * 2048], F32, tag="stg")
        for kc in range(4):
            preadds(stg, t0, TG, kc)
        th = tanh_pool.tile([128, TG * 2048], BF16, tag="th")
        nc.scalar.activation(th[:], stg[:], AF.Tanh)
        vreduce(th, t0, TG)
        if 4 <= g <= 7:
            # EW = enc @ Wtop, i.e. EW[s, o] = sum_h enc[s,h] Wtop[h,o];
            # score-independent, runs on the mostly-idle PE mid-loop (one
            # s-block per group so the DVE evacuation hides in the slack).
            sb = g - 4
            ep = ps_big.tile([128, 512], F32, tag="big")
            for hc in range(4):
                nc.tensor.matmul(ep[:],
                                 encT_sb[:, hc * 512 + sb * 128: hc * 512 + (sb + 1) * 128],
                                 wout_sb[:, hc * 512:(hc + 1) * 512],
                                 start=(hc == 0), stop=(hc == 3))
            nc.vector.tensor_copy(EW_sb[:, sb * 512:(sb + 1) * 512], ep[:])
        if g == 8:
            # ap2 = q @ Wbot, also score-independent.
            ap2 = ps_big.tile([64, 512], F32, tag="big")
            for hc in range(4):
                nc.tensor.matmul(ap2[:], qT_sb[:, hc * 64:(hc + 1) * 64],
                                 wout_sb[:, (4 + hc) * 512:(5 + hc) * 512],
                                 start=(hc == 0), stop=(hc == 3))
        if g == 9:
            nc.vector.tensor_copy(ap2_sb[:], ap2[:])

    # ---- softmax (transposed; no max subtraction) ----
    expT_sb = st.tile([128, 4 * 64], F32, tag="expT")
    nc.scalar.activation(expT_sb[:], scT[:], AF.Exp)

    dn = ps_small.tile([1, 64], F32, tag="small")
    for sb in range(4):
        nc.tensor.matmul(dn[:], ones_col[:], expT_sb[:, sb * 64:(sb + 1) * 64],
                         start=(sb == 0), stop=(sb == 3))
    d_sb = st.tile([1, 64], F32, tag="d")
    nc.vector.tensor_copy(d_sb[:], dn[:])
    r_sb = st.tile([1, 64], F32, tag="r")
    nc.vector.reciprocal(r_sb[:], d_sb[:])
    rp = ps_small.tile([64, 1], F32, tag="small")
    nc.tensor.matmul(rp[:], r_sb[:], ones_row[0:1, 0:1], start=True, stop=True)
    rT_sb = st.tile([64, 1], F32, tag="rT")
    nc.vector.tensor_copy(rT_sb[:], rp[:])

    # ---- output: tanh(r[t] * (expT.T @ EW) + ap2) ----
    ap1 = ps_big.tile([64, 512], F32, tag="big")
    for sb in range(4):
        nc.tensor.matmul(ap1[:], expT_sb[:, sb * 64:(sb + 1) * 64],
                         EW_sb[:, sb * 512:(sb + 1) * 512],
                         start=(sb == 0), stop=(sb == 3))
    sum_sb = st.tile([64, 512], F32, tag="sum")
    nc.vector.scalar_tensor_tensor(sum_sb[:], ap1[:], rT_sb[:], ap2_sb[:],
                                   op0=mybir.AluOpType.mult,
                                   op1=mybir.AluOpType.add)
    out_sb = st.tile([64, 512], F32, tag="out")
    nc.scalar.activation(out_sb[:], sum_sb[:], AF.Tanh)
    nc.sync.dma_start(io["out"][:], out_sb[:])


_NC_CACHE = None


def _get_nc():
    global _NC_CACHE
    if _NC_CACHE is None:
        nc = bacc.Bacc("TRN2", target_bir_lowering=False, debug=False,
                       num_devices=NCORES)
        io = {
            "encT": nc.dram_tensor("encT", [H, S], F32, kind="ExternalInput").ap(),
            "qT": nc.dram_tensor("qT", [H, TC], F32, kind="ExternalInput").ap(),
            "wh": nc.dram_tensor("wh", [H, H], F32, kind="ExternalInput").ap(),
            "ws": nc.dram_tensor("ws", [H, H], F32, kind="ExternalInput").ap(),
            "wout": nc.dram_tensor("wout", [2 * H, H], F32, kind="ExternalInput").ap(),
            "v4": nc.dram_tensor("v4", [128, 4], F32, kind="ExternalInput").ap(),
            "mneg": nc.dram_tensor("mneg", [4, 128], F32, kind="ExternalInput").ap(),
            "bones": nc.dram_tensor("bones", [4, 256], F32, kind="ExternalInput").ap(),
            "out": nc.dram_tensor("out", [TC, H], F32, kind="ExternalOutput").ap(),
        }
        with tile.TileContext(nc) as tc:
            with ExitStack() as ctx:
                _build_kernel(ctx, tc, io)
        nc.compile()
        _NC_CACHE = nc
    return _NC_CACHE


def _make_in_maps(query, encoder_outputs, src_lengths, W_h, W_s, v, W_out):
    f = lambda a: np.ascontiguousarray(np.asarray(a, dtype=np.float32))
    query, encoder_outputs = f(query), f(encoder_outputs)
    W_h, W_s, v, W_out = f(W_h), f(W_s), f(v), f(W_out)
    lens = np.asarray(src_lengths)
    v4 = np.ascontiguousarray(v.reshape(4, 128).T)  # v4[k, c] = v[c*128 + k]
    s_iota = np.arange(S)
    bones = np.kron(np.eye(4), np.ones((1, 64))).astype(np.float32)  # (4, 256)
    in_maps = []
    for j in range(NCORES):
        b, half = j // 2, j % 2
        mneg = np.where(s_iota < int(lens[b]), 0.0, MASK_NEG).astype(np.float32)
        in_maps.append({
            "encT": np.ascontiguousarray(encoder_outputs[b].T),
            "qT": np.ascontiguousarray(query[b, half * TC:(half + 1) * TC, :].T),
            "wh": W_h, "ws": W_s, "wout": W_out, "v4": v4,
            "mneg": mneg.reshape(4, 128), "bones": bones,
        })
    return in_maps


def kernel(query, encoder_outputs, src_lengths, W_h, W_s, v, W_out, _trace=False):
    nc = _get_nc()
    in_maps = _make_in_maps(query, encoder_outputs, src_lengths, W_h, W_s, v, W_out)
    res = run_bass_kernel_spmd(nc, in_maps, list(range(NCORES)), trace=_trace)
    out = np.empty((B, T, H), dtype=np.float32)
    for j in range(NCORES):
        b, half = j // 2, j % 2
        out[b, half * TC:(half + 1) * TC, :] = res.results[j]["out"]
    if _trace:
        return out, res
    return out
